# revision 11
# baseline (speedup 1.0000x reference)
"""Edge-softmax (GNN message passing) kernel for Trainium2 (Bass/Tile).

Strategy
--------
Host side (sharding/layout only, no float math):
  * Sort edges by destination node; assign each of the 8 NeuronCores a
    contiguous range of nodes (balanced by edge count) -> no cross-core
    reduction is needed.
  * Within a core, pack each node's edge run (padded to a multiple of 8
    with -1e9 sentinels) into 128*C "bins" (partition rows x chunks), so
    no node ever crosses a partition row or chunk boundary.
  * Emit three small block-level mask arrays that mark node-start /
    node-end blocks, used by the device-side segmented scans.

Device side (all float math):
  Per chunk [128, 8, Fc]:
    1. block max over groups of 8 edges            (DVE windowed reduce)
    2. forward segmented max-scan over block maxes (DVE tensor_tensor_scan)
    3. reverse segmented max-scan -> per-edge segment max
    4. scores = exp(logits - segmax)               (DVE sub + ACT exp)
    5. block sums of scores                        (DVE windowed reduce)
    6. forward segmented sum-scan -> per-node sums at node-end blocks

Host side (unshard): inverse-permute scores; pick per-node sums from the
scan output at each node's last block.
"""

import numpy as np

P = 128          # SBUF partitions (= bin rows)
H = 8            # attention heads
BLK = 8          # edges per block (dense reduce window)
NCORES = 8
C_CHUNKS = 6     # chunks along the free dim (= bins per partition row)
PADVAL = -1.0e9  # pad logit value (never wins max; exp() underflows to 0)
RESET = -1.0e38  # additive reset for the segmented max scans

SUB_ENGINE = "gpsimd"  # "vector" | "gpsimd" — engine for the subtract pass


# ----------------------------------------------------------------------------
# Host-side packing plan
# ----------------------------------------------------------------------------

def _greedy_bins(nblk, cap):
    """First-fit sequential packing of node block-runs into bins of `cap`
    blocks.  Returns (bin_id, blk0) per node and the number of bins used."""
    n = len(nblk)
    cb = np.cumsum(nblk)
    starts = [0]
    while True:
        s = starts[-1]
        base = cb[s - 1] if s > 0 else 0
        e = int(np.searchsorted(cb, base + cap, side="right"))
        assert e > s, "node larger than bin capacity"
        if e >= n:
            break
        starts.append(e)
    starts = np.asarray(starts, dtype=np.int64)
    bin_id = np.searchsorted(starts, np.arange(n), side="right") - 1
    base_blocks = np.where(starts > 0, cb[starts - 1], 0)
    cb_excl = cb - nblk
    blk0 = cb_excl - base_blocks[bin_id]
    return bin_id, blk0, len(starts)


def make_plan(dst, E, N, ncores=NCORES, C=C_CHUNKS):
    """Compute the full layout plan from the destination indices."""
    dst = np.asarray(dst).astype(np.int64).ravel()
    order = np.argsort(dst, kind="stable")
    dst_s = dst[order]
    counts = np.bincount(dst_s, minlength=N)

    nz = np.flatnonzero(counts)          # nodes with >=1 edge, ascending
    deg = counts[nz]
    nblk = (deg + BLK - 1) // BLK        # blocks per node

    # ---- core split: contiguous node ranges, balanced by edge count ----
    cume = np.cumsum(deg)
    targets = (np.arange(1, ncores) * E) // ncores
    splits = np.searchsorted(cume, targets)
    bounds = np.concatenate([[0], splits, [len(nz)]])

    # ---- per-core greedy packing; find the minimal common bin capacity ----
    nbins_total = P * C
    max_blocks = 0
    for c in range(ncores):
        max_blocks = max(max_blocks, int(nblk[bounds[c]:bounds[c + 1]].sum()))
    NBc = max(int(np.ceil(max_blocks / nbins_total)), int(nblk.max()))
    per_core = None
    while True:
        ok = True
        per_core = []
        for c in range(ncores):
            sl = slice(bounds[c], bounds[c + 1])
            bin_id, blk0, nbins = _greedy_bins(nblk[sl], NBc)
            if nbins > nbins_total:
                ok = False
                break
            per_core.append((bin_id, blk0))
        if ok:
            break
        NBc = NBc + max(1, NBc // 64)

    # ---- flatten per-node placement ----
    node_core = np.empty(len(nz), dtype=np.int64)
    node_bin = np.empty(len(nz), dtype=np.int64)
    node_blk0 = np.empty(len(nz), dtype=np.int64)
    for c in range(ncores):
        sl = slice(bounds[c], bounds[c + 1])
        node_core[sl] = c
        node_bin[sl] = per_core[c][0]
        node_blk0[sl] = per_core[c][1]

    node_p = node_bin % P                 # partition row
    node_c = node_bin // P                # chunk
    Fc = NBc * BLK
    FT = C * Fc
    NBT = C * NBc
    # column of the node's first edge within [FT]; block col within [NBT]
    node_col0 = node_c * Fc + node_blk0 * BLK
    node_gblk0 = node_c * NBc + node_blk0
    node_gblkL = node_gblk0 + nblk - 1

    # ---- per (sorted) edge placement ----
    cume_excl = cume - deg
    # rank of each sorted edge within its node
    rank = np.arange(E, dtype=np.int64) - np.repeat(cume_excl, deg)
    edge_node = np.repeat(np.arange(len(nz), dtype=np.int64), deg)
    e_core = node_core[edge_node]
    e_p = node_p[edge_node]
    e_f = node_col0[edge_node] + rank

    return dict(
        order=order, nz=nz, deg=deg, counts=counts,
        node_core=node_core, node_p=node_p,
        node_gblkL=node_gblkL, node_gblk0=node_gblk0,
        e_core=e_core, e_p=e_p, e_f=e_f,
        NBc=NBc, C=C, Fc=Fc, FT=FT, NBT=NBT, ncores=ncores,
    )


def pack_inputs(logits, plan):
    """Build per-core device input arrays from the plan."""
    logits = np.asarray(logits, dtype=np.float32).reshape(logits.shape[0], H)
    ncores, FT, NBT = plan["ncores"], plan["FT"], plan["NBT"]
    order = plan["order"]

    L = np.full((ncores, P, H, FT), PADVAL, dtype=np.float32)
    Lf = L.reshape(ncores * P, H, FT)
    r = plan["e_core"] * P + plan["e_p"]
    Lf[r, :, plan["e_f"]] = logits[order]

    mA = np.zeros((ncores, P, NBT), dtype=np.float32)
    mR = np.zeros((ncores, P, NBT), dtype=np.float32)
    mM = np.ones((ncores, P, NBT), dtype=np.float32)
    mAf = mA.reshape(ncores * P, NBT)
    mRf = mR.reshape(ncores * P, NBT)
    mMf = mM.reshape(ncores * P, NBT)
    rn = plan["node_core"] * P + plan["node_p"]
    mAf[rn, plan["node_gblk0"]] = RESET
    mRf[rn, plan["node_gblkL"]] = RESET
    mMf[rn, plan["node_gblk0"]] = 0.0
    return L, mA, mR, mM


def extract_outputs(S_list, PSB_list, plan, E, N):
    """Host-side unshard: scores back to original edge order; per-node sums."""
    S = np.stack(S_list)                          # [ncores, P, H, FT]
    PSB = np.stack(PSB_list)                      # [ncores, P, H, NBT]
    Sf = S.reshape(-1, H, plan["FT"])
    r = plan["e_core"] * P + plan["e_p"]
    scores_sorted = Sf[r, :, plan["e_f"]]         # [E, H]
    scores = np.empty((E, H), dtype=np.float32)
    scores[plan["order"]] = scores_sorted

    PSBf = PSB.reshape(-1, H, plan["NBT"])
    rn = plan["node_core"] * P + plan["node_p"]
    norm_nz = PSBf[rn, :, plan["node_gblkL"]]     # [n_nonzero, H]
    norm = np.zeros((N, H), dtype=np.float32)
    norm[plan["nz"]] = norm_nz
    return scores.reshape(E, H, 1), norm.reshape(N, H, 1)


# ----------------------------------------------------------------------------
# Device program
# ----------------------------------------------------------------------------

def _rev2(ap2d):
    """Reverse a free-contiguous 2D AP along its free dim."""
    import concourse.bass as bass
    (ps, pn), (fs, fn) = [list(x) for x in ap2d.ap]
    return bass.AP(ap2d.tensor, ap2d.offset + (fn - 1) * fs, [[ps, pn], [-fs, fn]])


def _mask3(mask_tile, c, NBc, rev=False):
    """Broadcast view of a [P, NBT] mask chunk as a (rep-8 x NBc) stream that
    matches a flattened [P, 8*NBc] scan operand (optionally reversed)."""
    import concourse.bass as bass
    ap = mask_tile[:, c * NBc:(c + 1) * NBc]
    (ps, pn), (fs, fn) = [list(x) for x in ap.ap]
    if rev:
        return bass.AP(ap.tensor, ap.offset + (fn - 1) * fs,
                       [[ps, pn], [0, H], [-fs, fn]])
    return bass.AP(ap.tensor, ap.offset, [[ps, pn], [0, H], [fs, fn]])


def _scan_raw(eng, out, d0, d1, initial, op0, op1):
    """tensor_tensor_scan without the 2D-shape restriction (d0 may be a
    broadcast 3D view whose stream order matches d1)."""
    import concourse.mybir as mybir
    return eng.add_instruction(
        mybir.InstTensorScalarPtr(
            name=eng.bass.get_next_instruction_name(),
            is_tensor_tensor_scan=True,
            is_scalar_tensor_tensor=True,
            op0=op0,
            op1=op1,
            ins=[eng.lower_ap(d0), eng.lower_ap_or_imm(initial),
                 eng.lower_ap(d1)],
            outs=[eng.lower_ap(out)],
        ))


def build_nc(NBc, C):
    import concourse.bass as bass
    import concourse.bacc as bacc
    import concourse.mybir as mybir
    from concourse.tile import TileContext

    f32 = mybir.dt.float32
    OP = mybir.AluOpType
    AX = mybir.AxisListType.X
    EXP = mybir.ActivationFunctionType.Exp

    Fc = NBc * BLK
    FT = C * Fc
    NBT = C * NBc

    nc = bacc.Bacc("TRN2", target_bir_lowering=False, num_devices=NCORES)
    L = nc.dram_tensor("L", [P, H, FT], f32, kind="ExternalInput")
    mA = nc.dram_tensor("mA", [P, NBT], f32, kind="ExternalInput")
    mR = nc.dram_tensor("mR", [P, NBT], f32, kind="ExternalInput")
    mM = nc.dram_tensor("mM", [P, NBT], f32, kind="ExternalInput")
    S = nc.dram_tensor("S", [P, H, FT], f32, kind="ExternalOutput")
    PSB = nc.dram_tensor("PSB", [P, H, NBT], f32, kind="ExternalOutput")

    with TileContext(nc) as tc:
        with (
            tc.tile_pool(name="masks", bufs=1) as mpool,
            tc.tile_pool(name="big", bufs=3) as bpool,
            tc.tile_pool(name="small", bufs=2) as spool,
            tc.tile_pool(name="out", bufs=1) as opool,
        ):
            # --- load host masks (read via broadcast views in the scans) ---
            mAt = mpool.tile([P, NBT], f32, name="mAt")
            nc.sync.dma_start(mAt[:], mA.ap())
            mRt = mpool.tile([P, NBT], f32, name="mRt")
            nc.sync.dma_start(mRt[:], mR.ap())
            mMt = mpool.tile([P, NBT], f32, name="mMt")
            nc.sync.dma_start(mMt[:], mM.ap())

            psb_full = opool.tile([P, H, NBT], f32, name="psb_full")

            for c in range(C):
                csl = slice(c * Fc, (c + 1) * Fc)
                Lc = bpool.tile([P, H, Fc], f32, tag="Lc", name=f"Lc{c}")
                nc.sync.dma_start(Lc[:], L.ap()[:, :, csl])
                Lc4 = Lc[:].rearrange("p h (nb k) -> p h nb k", k=BLK)
                Lc2 = Lc[:].rearrange("p h f -> p (h f)")

                # block max
                bm = spool.tile([P, H, NBc], f32, tag="bm", name=f"bm{c}")
                nc.vector.tensor_reduce(bm[:], Lc4, axis=AX, op=OP.max)
                bm2 = bm[:].rearrange("p h nb -> p (h nb)")

                # forward segmented max scan: state=(mA+state) max bm
                pm = spool.tile([P, H, NBc], f32, tag="pm", name=f"pm{c}")
                pm2 = pm[:].rearrange("p h nb -> p (h nb)")
                _scan_raw(nc.vector, pm2, _mask3(mAt, c, NBc), bm2,
                          RESET, OP.add, OP.max)

                # reverse segmented max scan over pm -> per-block segment max
                sx = spool.tile([P, H, NBc], f32, tag="sx", name=f"sx{c}")
                sx2 = sx[:].rearrange("p h nb -> p (h nb)")
                _scan_raw(nc.vector, _rev2(sx2), _mask3(mRt, c, NBc, rev=True),
                          _rev2(pm2), RESET, OP.add, OP.max)

                # scores = exp(L - segmax)
                sxb = sx[:].unsqueeze(3).broadcast_to([P, H, NBc, BLK])
                if SUB_ENGINE == "gpsimd":
                    nc.gpsimd.tensor_tensor(Lc4, Lc4, sxb, op=OP.subtract)
                else:
                    nc.vector.tensor_tensor(Lc4, Lc4, sxb, op=OP.subtract)
                nc.scalar.activation(Lc2, Lc2, EXP)

                # block sums of scores
                bs = spool.tile([P, H, NBc], f32, tag="bs", name=f"bs{c}")
                nc.vector.tensor_reduce(bs[:], Lc4, axis=AX, op=OP.add)
                bs2 = bs[:].rearrange("p h nb -> p (h nb)")

                # forward segmented sum scan: state=(mM*state)+bs
                psb = spool.tile([P, H, NBc], f32, tag="psb", name=f"psb{c}")
                psb2 = psb[:].rearrange("p h nb -> p (h nb)")
                _scan_raw(nc.vector, psb2, _mask3(mMt, c, NBc), bs2,
                          0.0, OP.mult, OP.add)
                nc.scalar.copy(psb_full[:, :, c * NBc:(c + 1) * NBc], psb[:])

                # write scores
                nc.sync.dma_start(S.ap()[:, :, csl], Lc[:])

            nc.sync.dma_start(PSB.ap(), psb_full[:])

    nc.compile()
    return nc


# ----------------------------------------------------------------------------
# Entry point
# ----------------------------------------------------------------------------

_TRACE = False       # set by test.py to capture an NTFF profile
_LAST_RESULT = None  # BassKernelResults of the last run (for test.py)


def kernel(logits, dst, num_nodes):
    global _LAST_RESULT
    from concourse.bass_utils import run_bass_kernel_spmd

    logits = np.asarray(logits, dtype=np.float32)
    dst = np.asarray(dst)
    E = logits.shape[0]
    N = int(num_nodes)

    plan = make_plan(dst, E, N)
    L, mA, mR, mM = pack_inputs(logits, plan)

    nc = build_nc(plan["NBc"], plan["C"])
    in_maps = [
        {"L": L[i], "mA": mA[i], "mR": mR[i], "mM": mM[i]}
        for i in range(NCORES)
    ]
    res = run_bass_kernel_spmd(
        nc, in_maps, core_ids=list(range(NCORES)), trace=_TRACE)
    _LAST_RESULT = res
    S_list = [r["S"] for r in res.results]
    PSB_list = [r["PSB"] for r in res.results]

    scores, norm = extract_outputs(S_list, PSB_list, plan, E, N)
    return scores, norm


# revision 13
# speedup vs baseline: 1.1307x; 1.1307x over previous
"""Edge-softmax (GNN message passing) kernel for Trainium2 (Bass/Tile).

Strategy
--------
Host side (sharding/layout only, no float math):
  * Sort edges by destination node; assign each of the 8 NeuronCores a
    contiguous range of nodes (balanced by edge count) -> no cross-core
    reduction is needed.
  * Within a core, pack each node's edge run (padded to a multiple of 8
    with -1e9 sentinels) into 128*C "bins" (partition rows x chunks), so
    no node ever crosses a partition row or chunk boundary.
  * Emit three small block-level mask arrays that mark node-start /
    node-end blocks, used by the device-side segmented scans.

Device side (all float math):
  Per chunk [128, 8, Fc]:
    1. block max over groups of 8 edges            (DVE windowed reduce)
    2. forward segmented max-scan over block maxes (DVE tensor_tensor_scan)
    3. reverse segmented max-scan -> per-edge segment max
    4. scores = exp(logits - segmax)               (DVE sub + ACT exp)
    5. block sums of scores                        (DVE windowed reduce)
    6. forward segmented sum-scan -> per-node sums at node-end blocks

Host side (unshard): inverse-permute scores; pick per-node sums from the
scan output at each node's last block.
"""

import numpy as np

P = 128          # SBUF partitions (= bin rows)
H = 8            # attention heads
BLK = 8          # edges per block (dense reduce window)
NCORES = 8
C_CHUNKS = 6     # chunks along the free dim (= bins per partition row)
PADVAL = -1.0e9  # pad logit value (never wins max; exp() underflows to 0)
RESET = -1.0e38  # additive reset for the segmented max scans

SUB_ENGINE = "gpsimd"  # "vector" | "gpsimd" — engine for the subtract pass


# ----------------------------------------------------------------------------
# Host-side packing plan
# ----------------------------------------------------------------------------

def _greedy_bins(nblk, cap):
    """First-fit sequential packing of node block-runs into bins of `cap`
    blocks.  Returns (bin_id, blk0) per node and the number of bins used."""
    n = len(nblk)
    cb = np.cumsum(nblk)
    starts = [0]
    while True:
        s = starts[-1]
        base = cb[s - 1] if s > 0 else 0
        e = int(np.searchsorted(cb, base + cap, side="right"))
        assert e > s, "node larger than bin capacity"
        if e >= n:
            break
        starts.append(e)
    starts = np.asarray(starts, dtype=np.int64)
    bin_id = np.searchsorted(starts, np.arange(n), side="right") - 1
    base_blocks = np.where(starts > 0, cb[starts - 1], 0)
    cb_excl = cb - nblk
    blk0 = cb_excl - base_blocks[bin_id]
    return bin_id, blk0, len(starts)


def make_plan(dst, E, N, ncores=NCORES, C=C_CHUNKS):
    """Compute the full layout plan from the destination indices."""
    dst = np.asarray(dst).astype(np.int64).ravel()
    order = np.argsort(dst, kind="stable")
    dst_s = dst[order]
    counts = np.bincount(dst_s, minlength=N)

    nz = np.flatnonzero(counts)          # nodes with >=1 edge, ascending
    deg = counts[nz]
    nblk = (deg + BLK - 1) // BLK        # blocks per node

    # ---- core split: contiguous node ranges, balanced by edge count ----
    cume = np.cumsum(deg)
    targets = (np.arange(1, ncores) * E) // ncores
    splits = np.searchsorted(cume, targets)
    bounds = np.concatenate([[0], splits, [len(nz)]])

    # ---- per-core greedy packing; find the minimal common bin capacity ----
    nbins_total = P * C
    max_blocks = 0
    for c in range(ncores):
        max_blocks = max(max_blocks, int(nblk[bounds[c]:bounds[c + 1]].sum()))
    NBc = max(int(np.ceil(max_blocks / nbins_total)), int(nblk.max()))
    per_core = None
    while True:
        ok = True
        per_core = []
        for c in range(ncores):
            sl = slice(bounds[c], bounds[c + 1])
            bin_id, blk0, nbins = _greedy_bins(nblk[sl], NBc)
            if nbins > nbins_total:
                ok = False
                break
            per_core.append((bin_id, blk0))
        if ok:
            break
        NBc = NBc + max(1, NBc // 64)

    # ---- flatten per-node placement ----
    node_core = np.empty(len(nz), dtype=np.int64)
    node_bin = np.empty(len(nz), dtype=np.int64)
    node_blk0 = np.empty(len(nz), dtype=np.int64)
    for c in range(ncores):
        sl = slice(bounds[c], bounds[c + 1])
        node_core[sl] = c
        node_bin[sl] = per_core[c][0]
        node_blk0[sl] = per_core[c][1]

    node_p = node_bin % P                 # partition row
    node_c = node_bin // P                # chunk
    Fc = NBc * BLK
    FT = C * Fc
    NBT = C * NBc
    # column of the node's first edge within [FT]; block col within [NBT]
    node_col0 = node_c * Fc + node_blk0 * BLK
    node_gblk0 = node_c * NBc + node_blk0
    node_gblkL = node_gblk0 + nblk - 1

    # ---- per (sorted) edge placement ----
    cume_excl = cume - deg
    # rank of each sorted edge within its node
    rank = np.arange(E, dtype=np.int64) - np.repeat(cume_excl, deg)
    edge_node = np.repeat(np.arange(len(nz), dtype=np.int64), deg)
    e_core = node_core[edge_node]
    e_p = node_p[edge_node]
    e_f = node_col0[edge_node] + rank

    return dict(
        order=order, nz=nz, deg=deg, counts=counts,
        node_core=node_core, node_p=node_p,
        node_gblkL=node_gblkL, node_gblk0=node_gblk0,
        e_core=e_core, e_p=e_p, e_f=e_f,
        NBc=NBc, C=C, Fc=Fc, FT=FT, NBT=NBT, ncores=ncores,
    )


def pack_inputs(logits, plan):
    """Build per-core device input arrays from the plan."""
    logits = np.asarray(logits, dtype=np.float32).reshape(logits.shape[0], H)
    ncores, FT, NBT = plan["ncores"], plan["FT"], plan["NBT"]
    order = plan["order"]

    L = np.full((ncores, P, H, FT), PADVAL, dtype=np.float32)
    Lf = L.reshape(ncores * P, H, FT)
    r = plan["e_core"] * P + plan["e_p"]
    Lf[r, :, plan["e_f"]] = logits[order]

    mA = np.zeros((ncores, P, NBT), dtype=np.float32)
    mR = np.zeros((ncores, P, NBT), dtype=np.float32)
    mM = np.ones((ncores, P, NBT), dtype=np.float32)
    mAf = mA.reshape(ncores * P, NBT)
    mRf = mR.reshape(ncores * P, NBT)
    mMf = mM.reshape(ncores * P, NBT)
    rn = plan["node_core"] * P + plan["node_p"]
    mAf[rn, plan["node_gblk0"]] = RESET
    mRf[rn, plan["node_gblkL"]] = RESET
    mMf[rn, plan["node_gblk0"]] = 0.0
    return L, mA, mR, mM


def extract_outputs(S_list, PSB_list, plan, E, N):
    """Host-side unshard: scores back to original edge order; per-node sums."""
    S = np.stack(S_list)                          # [ncores, P, H, FT]
    PSB = np.stack(PSB_list)                      # [ncores, P, H, NBT]
    Sf = S.reshape(-1, H, plan["FT"])
    r = plan["e_core"] * P + plan["e_p"]
    scores_sorted = Sf[r, :, plan["e_f"]]         # [E, H]
    scores = np.empty((E, H), dtype=np.float32)
    scores[plan["order"]] = scores_sorted

    PSBf = PSB.reshape(-1, H, plan["NBT"])
    rn = plan["node_core"] * P + plan["node_p"]
    norm_nz = PSBf[rn, :, plan["node_gblkL"]]     # [n_nonzero, H]
    norm = np.zeros((N, H), dtype=np.float32)
    norm[plan["nz"]] = norm_nz
    return scores.reshape(E, H, 1), norm.reshape(N, H, 1)


# ----------------------------------------------------------------------------
# Device program
# ----------------------------------------------------------------------------

def _rev2(ap2d):
    """Reverse a free-contiguous 2D AP along its free dim."""
    import concourse.bass as bass
    (ps, pn), (fs, fn) = [list(x) for x in ap2d.ap]
    return bass.AP(ap2d.tensor, ap2d.offset + (fn - 1) * fs, [[ps, pn], [-fs, fn]])


def _mask3(mask_tile, c, NBc, rev=False):
    """Broadcast view of a [P, NBT] mask chunk as a (rep-8 x NBc) stream that
    matches a flattened [P, 8*NBc] scan operand (optionally reversed)."""
    import concourse.bass as bass
    ap = mask_tile[:, c * NBc:(c + 1) * NBc]
    (ps, pn), (fs, fn) = [list(x) for x in ap.ap]
    if rev:
        return bass.AP(ap.tensor, ap.offset + (fn - 1) * fs,
                       [[ps, pn], [0, H], [-fs, fn]])
    return bass.AP(ap.tensor, ap.offset, [[ps, pn], [0, H], [fs, fn]])


def _scan_raw(eng, out, d0, d1, initial, op0, op1):
    """tensor_tensor_scan without the 2D-shape restriction (d0 may be a
    broadcast 3D view whose stream order matches d1)."""
    import concourse.mybir as mybir
    return eng.add_instruction(
        mybir.InstTensorScalarPtr(
            name=eng.bass.get_next_instruction_name(),
            is_tensor_tensor_scan=True,
            is_scalar_tensor_tensor=True,
            op0=op0,
            op1=op1,
            ins=[eng.lower_ap(d0), eng.lower_ap_or_imm(initial),
                 eng.lower_ap(d1)],
            outs=[eng.lower_ap(out)],
        ))


def build_nc(NBc, C):
    import concourse.bass as bass
    import concourse.bacc as bacc
    import concourse.mybir as mybir
    from concourse.tile import TileContext

    f32 = mybir.dt.float32
    OP = mybir.AluOpType
    AX = mybir.AxisListType.X
    EXP = mybir.ActivationFunctionType.Exp

    Fc = NBc * BLK
    FT = C * Fc
    NBT = C * NBc

    nc = bacc.Bacc("TRN2", target_bir_lowering=False, num_devices=NCORES)
    L = nc.dram_tensor("L", [P, H, FT], f32, kind="ExternalInput")
    mA = nc.dram_tensor("mA", [P, NBT], f32, kind="ExternalInput")
    mR = nc.dram_tensor("mR", [P, NBT], f32, kind="ExternalInput")
    mM = nc.dram_tensor("mM", [P, NBT], f32, kind="ExternalInput")
    S = nc.dram_tensor("S", [P, H, FT], f32, kind="ExternalOutput")
    PSB = nc.dram_tensor("PSB", [P, H, NBT], f32, kind="ExternalOutput")

    with TileContext(nc) as tc:
        with (
            tc.tile_pool(name="masks", bufs=1) as mpool,
            tc.tile_pool(name="big", bufs=4) as bpool,
            tc.tile_pool(name="small", bufs=2) as spool,
            tc.tile_pool(name="out", bufs=1) as opool,
        ):
            # --- load host masks (read via broadcast views in the scans) ---
            mAt = mpool.tile([P, NBT], f32, name="mAt")
            nc.sync.dma_start(mAt[:], mA.ap())
            mRt = mpool.tile([P, NBT], f32, name="mRt")
            nc.sync.dma_start(mRt[:], mR.ap())
            mMt = mpool.tile([P, NBT], f32, name="mMt")
            nc.sync.dma_start(mMt[:], mM.ap())

            psb_full = opool.tile([P, H, NBT], f32, name="psb_full")

            # software-pipelined emission: stage A (load + max) runs ahead,
            # stage B (sub + exp) one chunk behind, stage C (sums + store)
            # two behind — keeps every engine's in-order stream unblocked.
            state = {}

            def stage_a(c):
                csl = slice(c * Fc, (c + 1) * Fc)
                Lc = bpool.tile([P, H, Fc], f32, tag="Lc", name=f"Lc{c}")
                nc.sync.dma_start(Lc[:], L.ap()[:, :, csl])
                Lc4 = Lc[:].rearrange("p h (nb k) -> p h nb k", k=BLK)

                # block max
                bm = spool.tile([P, H, NBc], f32, tag="bm", name=f"bm{c}",
                                bufs=2)
                nc.vector.tensor_reduce(bm[:], Lc4, axis=AX, op=OP.max)
                bm2 = bm[:].rearrange("p h nb -> p (h nb)")

                # forward segmented max scan: state=(mA+state) max bm
                pm = spool.tile([P, H, NBc], f32, tag="pm", name=f"pm{c}",
                                bufs=2)
                pm2 = pm[:].rearrange("p h nb -> p (h nb)")
                _scan_raw(nc.vector, pm2, _mask3(mAt, c, NBc), bm2,
                          RESET, OP.add, OP.max)

                # reverse segmented max scan over pm -> per-block segment max
                sx = spool.tile([P, H, NBc], f32, tag="sx", name=f"sx{c}",
                                bufs=3)
                sx2 = sx[:].rearrange("p h nb -> p (h nb)")
                _scan_raw(nc.vector, _rev2(sx2), _mask3(mRt, c, NBc, rev=True),
                          _rev2(pm2), RESET, OP.add, OP.max)
                state[c] = (Lc, Lc4, sx)

            def stage_b(c):
                Lc, Lc4, sx = state[c]
                sxb = sx[:].unsqueeze(3).broadcast_to([P, H, NBc, BLK])
                if SUB_ENGINE == "gpsimd":
                    nc.gpsimd.tensor_tensor(Lc4, Lc4, sxb, op=OP.subtract)
                else:
                    nc.vector.tensor_tensor(Lc4, Lc4, sxb, op=OP.subtract)
                Lc2 = Lc[:].rearrange("p h f -> p (h f)")
                nc.scalar.activation(Lc2, Lc2, EXP)

            def stage_c(c):
                Lc, Lc4, sx = state[c]
                csl = slice(c * Fc, (c + 1) * Fc)
                bs = spool.tile([P, H, NBc], f32, tag="bs", name=f"bs{c}",
                                bufs=2)
                nc.vector.tensor_reduce(bs[:], Lc4, axis=AX, op=OP.add)
                bs2 = bs[:].rearrange("p h nb -> p (h nb)")

                # forward segmented sum scan: state=(mM*state)+bs
                psb = spool.tile([P, H, NBc], f32, tag="psb", name=f"psb{c}",
                                bufs=2)
                psb2 = psb[:].rearrange("p h nb -> p (h nb)")
                _scan_raw(nc.vector, psb2, _mask3(mMt, c, NBc), bs2,
                          0.0, OP.mult, OP.add)
                nc.scalar.copy(psb_full[:, :, c * NBc:(c + 1) * NBc], psb[:])

                # write scores
                nc.sync.dma_start(S.ap()[:, :, csl], Lc[:])

            for c in range(C + 2):
                if c < C:
                    stage_a(c)
                if 0 <= c - 1 < C:
                    stage_b(c - 1)
                if 0 <= c - 2 < C:
                    stage_c(c - 2)

            nc.sync.dma_start(PSB.ap(), psb_full[:])

    nc.compile()
    return nc


# ----------------------------------------------------------------------------
# Entry point
# ----------------------------------------------------------------------------

_TRACE = False       # set by test.py to capture an NTFF profile
_LAST_RESULT = None  # BassKernelResults of the last run (for test.py)


def kernel(logits, dst, num_nodes):
    global _LAST_RESULT
    from concourse.bass_utils import run_bass_kernel_spmd

    logits = np.asarray(logits, dtype=np.float32)
    dst = np.asarray(dst)
    E = logits.shape[0]
    N = int(num_nodes)

    plan = make_plan(dst, E, N)
    L, mA, mR, mM = pack_inputs(logits, plan)

    nc = build_nc(plan["NBc"], plan["C"])
    in_maps = [
        {"L": L[i], "mA": mA[i], "mR": mR[i], "mM": mM[i]}
        for i in range(NCORES)
    ]
    res = run_bass_kernel_spmd(
        nc, in_maps, core_ids=list(range(NCORES)), trace=_TRACE)
    _LAST_RESULT = res
    S_list = [r["S"] for r in res.results]
    PSB_list = [r["PSB"] for r in res.results]

    scores, norm = extract_outputs(S_list, PSB_list, plan, E, N)
    return scores, norm
